# revision 13
# baseline (speedup 1.0000x reference)
"""Corr2Cost sampling kernel for 8 TRN2 NeuronCores.

Math: out[b,c,k,i,j] = lerp of corr[b,c,:,i,j] at depth (j + k - maxdisp)
(is_ux=1) with zero padding outside [0, D-1].  For integer maxdisp the
displacements linspace(-md, md, 2*md+1) are exact integers, so the lerp
weight is exactly 0 and the op is a pure masked integer gather:

    out[b,c,k,i,j] = corr[b,c, j+k-md, i, j]   if 0 <= j+k-md < D else 0

Sharding: data-parallel over the 16 (b,c) pairs -> 2 pairs per core; no
cross-core communication.

Layout strategy (all DMA at max descriptor efficiency):
  - only the band |d - j| <= md of corr is ever read (d = j+k-md, k in
    [0, 2md]), so the host packs the diagonal band per pair:
        xb[i, d*Kb + m] = corr[d, i, d-md+m],  m in [0, Kb), Kb = 2md+1
    (junk where the source j is out of range -- those slots are never
    read).  The per-pair load is then 96 partitions x Kb*D*4 contiguous
    bytes, 21% fewer than the full tensor;
  - the gather for output row k is the constant-stride-Kb slice
        A[i, (j+k-md)*Kb + (2md-k)] -> one strided tensor_copy per k
    into O[i, kk*W + j];
  - O is stored as (i, k, j) slabs (per-partition contiguous), host
    post-transposes to (k, i, j).  Border regions are zeroed by one
    rectangular over-memset per k-chunk;
  - loads go on the SP HWDGE ring (nc.sync), stores on the ACT ring
    (nc.scalar): the two streams are byte-balanced and can overlap.
"""

import numpy as np

B, C, D, H, W = 8, 2, 128, 96, 128
N_CORES = 8
PAIRS = B * C  # 16
PAIRS_PER_CORE = PAIRS // N_CORES  # 2

_NC_CACHE = {}


def _k_chunks(K, small_first=False):
    """Split [0, K) into ~4 chunks; small_first prepends a tiny chunk so the
    pair's first store is ready as soon as its ring frees up."""
    if K < 8:
        return [(0, K)]
    bounds = [0]
    if small_first:
        bounds.append(min(6, K // 2))
    rest = K - bounds[-1]
    n = 4
    start = bounds[-1]
    bounds += [start + round(i * rest / n) for i in range(1, n + 1)]
    return [(bounds[i], bounds[i + 1]) for i in range(len(bounds) - 1)]


def _build_bass(md: int, reps: int = 1):
    """Build + compile the per-core Bass graph for is_ux=1, given maxdisp.

    reps > 1 wraps the body in a hardware For_i loop (timing harness only).
    """
    import concourse.bacc as bacc
    import concourse.mybir as mybir
    import concourse.tile as tile

    K = 2 * md + 1
    f32 = mybir.dt.float32

    nc = bacc.Bacc("TRN2", target_bir_lowering=False, debug=False)
    x = nc.dram_tensor("x", [PAIRS_PER_CORE, H, D * K], f32, kind="ExternalInput")
    y = nc.dram_tensor("y", [PAIRS_PER_CORE, H, K * W], f32, kind="ExternalOutput")

    def body(tc, apool, opool):
        # Loads first: each pair's band is split into two d-halves, one per
        # HWDGE ring, so every pair is fully resident by ~bytes/(2 rings).
        hd = D // 2
        tiles = []
        for p in range(PAIRS_PER_CORE):
            a = apool.tile([H, D * K], f32)
            a3 = a[:].rearrange("h (d m) -> h d m", m=K)
            x3 = x[p].rearrange("h (d m) -> h d m", m=K)
            lo_eng = nc.sync if p % 2 == 0 else nc.scalar
            hi_eng = nc.scalar if p % 2 == 0 else nc.sync
            lo_eng.dma_start(out=a3[:, :hd], in_=x3[:, :hd])
            hi_eng.dma_start(out=a3[:, hd:], in_=x3[:, hd:])
            tiles.append(a)
        for p in range(PAIRS_PER_CORE):
            # stores alternate rings per pair; copies alternate engines
            st_eng = nc.scalar if p % 2 == 0 else nc.sync
            cp_eng = nc.vector if p % 2 == 0 else nc.gpsimd
            a = tiles[p]
            for (k0, k1) in _k_chunks(K, small_first=(p % 2 == 1)):
                ck = k1 - k0
                o = opool.tile([H, ck * W], f32)
                o3 = o[:].rearrange("p (kk j) -> p kk j", j=W)
                # rectangular over-memset covering the masked border of
                # this chunk (copies below overwrite the valid part)
                lmax = max(0, md - k0)          # left border width at k0
                rmax = max(0, k1 - 1 - md)      # right border width at k1-1
                if lmax > 0:
                    cp_eng.memset(o3[:, :, 0:lmax], 0.0)
                if rmax > 0:
                    cp_eng.memset(o3[:, :, W - rmax : W], 0.0)
                for k in range(k0, k1):
                    j0 = max(0, md - k)
                    j1 = min(W - 1, D - 1 + md - k)
                    n = j1 - j0 + 1
                    # band layout: element (d=j+k-md, m=2md-k) at flat
                    # d*K + m  ->  stride K in j
                    off0 = (j0 + k - md) * K + (2 * md - k)
                    kk = k - k0
                    cp_eng.tensor_copy(
                        o3[:, kk, j0 : j1 + 1],
                        a[:, off0 : off0 + (n - 1) * K + 1 : K],
                    )
                st_eng.dma_start(out=y[p][:, k0 * W : k1 * W], in_=o[:])

    with tile.TileContext(nc) as tc:
        with (
            tc.tile_pool(name="a", bufs=2) as apool,
            tc.tile_pool(name="o", bufs=6) as opool,
        ):
            if reps == 1:
                body(tc, apool, opool)
            else:
                with tc.For_i(0, reps, 1):
                    body(tc, apool, opool)

    nc.compile()
    return nc


def _get_nc(md: int, reps: int = 1):
    key = (md, reps)
    if key not in _NC_CACHE:
        _NC_CACHE[key] = _build_bass(md, reps)
    return _NC_CACHE[key]


def _numpy_ref(corr, maxdisp, is_ux):
    """Exact numpy replication of the reference (fallback path)."""
    corr = np.asarray(corr)
    b, c, d_, h, w = corr.shape
    K = 2 * maxdisp + 1
    dx = np.linspace(-float(maxdisp), float(maxdisp), K).astype(np.float32)
    if is_ux:
        base = np.broadcast_to(np.arange(w, dtype=np.float32)[None, :], (h, w))
    else:
        base = np.broadcast_to(np.arange(h, dtype=np.float32)[:, None], (h, w))
    pos = base[None, :, :] + dx[:, None, None]
    i0f = np.floor(pos)
    w1 = (pos - i0f).astype(corr.dtype)
    i0 = i0f.astype(np.int32)
    i1 = i0 + 1
    m0 = ((i0 >= 0) & (i0 < d_)).astype(corr.dtype)
    m1 = ((i1 >= 0) & (i1 < d_)).astype(corr.dtype)
    idx0 = np.clip(i0, 0, d_ - 1)[None, None]
    idx1 = np.clip(i1, 0, d_ - 1)[None, None]
    g0 = np.take_along_axis(corr, np.broadcast_to(idx0, (b, c, K, h, w)), axis=2)
    g1 = np.take_along_axis(corr, np.broadcast_to(idx1, (b, c, K, h, w)), axis=2)
    return g0 * ((1.0 - w1) * m0)[None, None] + g1 * (w1 * m1)[None, None]


def _run_on_device(corr, md: int, reps: int = 1):
    from concourse.bass_utils import run_bass_kernel_spmd

    K = 2 * md + 1
    nc = _get_nc(md, reps)
    # (B, C, D, H, W) -> (16, D, H, W) -> (16, H, D, W), then pack the
    # diagonal band: xb[p, i, d, m] = corr[p, d, i, d-md+m]
    flat = np.asarray(corr).reshape(PAIRS, D, H, W)
    xt = flat.transpose(0, 2, 1, 3)  # (16, H, D, W) view
    xb = np.zeros((PAIRS, H, D, K), np.float32)
    for d in range(D):
        jlo = max(0, d - md)
        jhi = min(W, d + md + 1)
        mlo = jlo - (d - md)
        xb[:, :, d, mlo : mlo + (jhi - jlo)] = xt[:, :, d, jlo:jhi]
    xb = xb.reshape(PAIRS, H, D * K)
    in_maps = [
        {"x": xb[PAIRS_PER_CORE * c : PAIRS_PER_CORE * (c + 1)]}
        for c in range(N_CORES)
    ]
    res = run_bass_kernel_spmd(nc, in_maps, core_ids=list(range(N_CORES)))
    out = np.concatenate([res.results[c]["y"] for c in range(N_CORES)], axis=0)
    # (16, H, K*W) -> (16, H, K, W) -> (16, K, H, W) -> (B, C, K, H, W)
    out = out.reshape(PAIRS, H, K, W).transpose(0, 2, 1, 3)
    out = np.ascontiguousarray(out).reshape(B, C, K, H, W)
    return out, res


def kernel(corr, maxdisp, is_ux):
    corr = np.asarray(corr)
    md = int(maxdisp)
    ux = int(is_ux)
    if ux != 1 or md < 1 or md > 127 or corr.shape != (B, C, D, H, W):
        return _numpy_ref(corr, md, ux).astype(corr.dtype)
    out, _ = _run_on_device(corr, md)
    return out


# revision 15
# speedup vs baseline: 1.3160x; 1.3160x over previous
"""Corr2Cost sampling kernel for 8 TRN2 NeuronCores.

Math: out[b,c,k,i,j] = lerp of corr[b,c,:,i,j] at depth (j + k - maxdisp)
(is_ux=1) with zero padding outside [0, D-1].  For integer maxdisp the
displacements linspace(-md, md, 2*md+1) are exact integers, so the lerp
weight is exactly 0 and the op is a pure masked integer gather:

    out[b,c,k,i,j] = corr[b,c, j+k-md, i, j]   if 0 <= j+k-md < D else 0

Sharding: data-parallel over the 16 (b,c) pairs -> 2 pairs per core; no
cross-core communication.

Layout strategy (all DMA at max descriptor efficiency):
  - only the band |d - j| <= md of corr is ever read (d = j+k-md, k in
    [0, 2md]), so the host packs the diagonal band per pair:
        xb[i, d*Kb + m] = corr[d, i, d-md+m],  m in [0, Kb), Kb = 2md+1
    (junk where the source j is out of range -- those slots are never
    read).  The per-pair load is then 96 partitions x Kb*D*4 contiguous
    bytes, 21% fewer than the full tensor;
  - the gather for output row k is the constant-stride-Kb slice
        A[i, (j+k-md)*Kb + (2md-k)] -> one strided tensor_copy per k
    into O[i, kk*W + j];
  - O is stored as (i, k, j) slabs (per-partition contiguous), host
    post-transposes to (k, i, j).  Border regions are zeroed by one
    rectangular over-memset per k-chunk;
  - loads go on the SP HWDGE ring (nc.sync), stores on the ACT ring
    (nc.scalar): the two streams are byte-balanced and can overlap.
"""

import numpy as np

B, C, D, H, W = 8, 2, 128, 96, 128
N_CORES = 8
PAIRS = B * C  # 16
PAIRS_PER_CORE = PAIRS // N_CORES  # 2

_NC_CACHE = {}


def _k_chunks(K, small_first=False):
    """Split [0, K) into ~4 chunks; small_first prepends a tiny chunk so the
    pair's first store is ready as soon as its ring frees up."""
    if K < 8:
        return [(0, K)]
    bounds = [0]
    if small_first:
        bounds.append(min(6, K // 2))
    rest = K - bounds[-1]
    n = 4
    start = bounds[-1]
    bounds += [start + round(i * rest / n) for i in range(1, n + 1)]
    return [(bounds[i], bounds[i + 1]) for i in range(len(bounds) - 1)]


def _build_bass(md: int, reps: int = 1):
    """Build + compile the per-core Bass graph for is_ux=1, given maxdisp.

    reps > 1 wraps the body in a hardware For_i loop (timing harness only).
    """
    import concourse.bacc as bacc
    import concourse.mybir as mybir
    import concourse.tile as tile

    K = 2 * md + 1
    f32 = mybir.dt.float32

    nc = bacc.Bacc("TRN2", target_bir_lowering=False, debug=False)
    x = nc.dram_tensor("x", [PAIRS_PER_CORE, H, D * K], f32, kind="ExternalInput")
    y = nc.dram_tensor("y", [PAIRS_PER_CORE, H, K * W], f32, kind="ExternalOutput")

    # Measured on this terminal: only exact-128-partition DMA streams reach
    # ~370 GB/s; 96-partition shapes get ~176, and DMAs running CONCURRENTLY
    # on both HWDGE rings degrade ~2x below running serially on one ring.
    # So: flatten the 2 pairs to 192 rows, tile as 128 + 64 rows, and issue
    # every DMA serially on the single SP ring in stream order.
    ROWS = PAIRS_PER_CORE * H  # 192
    RA = 128                   # rows in the fast tile
    RB = ROWS - RA             # 64

    def body(tc, apool, opool):
        x_flat = x[:].rearrange("p h f -> (p h) f")   # (192, D*K)
        y_flat = y[:].rearrange("p h f -> (p h) f")   # (192, K*W)
        ta = apool.tile([RA, D * K], f32)
        tb = apool.tile([RB, D * K], f32)
        nc.sync.dma_start(out=ta[:], in_=x_flat[0:RA])
        nc.sync.dma_start(out=tb[:], in_=x_flat[RA:ROWS])
        for a, rows, r0, cp_eng in (
            (ta, RA, 0, nc.vector),
            (tb, RB, RA, nc.gpsimd),
        ):
            for (k0, k1) in _k_chunks(K):
                ck = k1 - k0
                o = opool.tile([rows, ck * W], f32)
                o3 = o[:].rearrange("q (kk j) -> q kk j", j=W)
                # rectangular over-memset covering the masked border of
                # this chunk (copies below overwrite the valid part)
                lmax = max(0, md - k0)          # left border width at k0
                rmax = max(0, k1 - 1 - md)      # right border width at k1-1
                if lmax > 0:
                    cp_eng.memset(o3[:, :, 0:lmax], 0.0)
                if rmax > 0:
                    cp_eng.memset(o3[:, :, W - rmax : W], 0.0)
                for k in range(k0, k1):
                    j0 = max(0, md - k)
                    j1 = min(W - 1, D - 1 + md - k)
                    n = j1 - j0 + 1
                    # band layout: element (d=j+k-md, m=2md-k) at flat
                    # d*K + m  ->  stride K in j
                    off0 = (j0 + k - md) * K + (2 * md - k)
                    kk = k - k0
                    cp_eng.tensor_copy(
                        o3[:, kk, j0 : j1 + 1],
                        a[:, off0 : off0 + (n - 1) * K + 1 : K],
                    )
                nc.sync.dma_start(
                    out=y_flat[r0 : r0 + rows, k0 * W : k1 * W], in_=o[:]
                )

    with tile.TileContext(nc) as tc:
        with (
            tc.tile_pool(name="a", bufs=1) as apool,
            tc.tile_pool(name="o", bufs=6) as opool,
        ):
            if reps == 1:
                body(tc, apool, opool)
            else:
                with tc.For_i(0, reps, 1):
                    body(tc, apool, opool)

    nc.compile()
    return nc


def _get_nc(md: int, reps: int = 1):
    key = (md, reps)
    if key not in _NC_CACHE:
        _NC_CACHE[key] = _build_bass(md, reps)
    return _NC_CACHE[key]


def _numpy_ref(corr, maxdisp, is_ux):
    """Exact numpy replication of the reference (fallback path)."""
    corr = np.asarray(corr)
    b, c, d_, h, w = corr.shape
    K = 2 * maxdisp + 1
    dx = np.linspace(-float(maxdisp), float(maxdisp), K).astype(np.float32)
    if is_ux:
        base = np.broadcast_to(np.arange(w, dtype=np.float32)[None, :], (h, w))
    else:
        base = np.broadcast_to(np.arange(h, dtype=np.float32)[:, None], (h, w))
    pos = base[None, :, :] + dx[:, None, None]
    i0f = np.floor(pos)
    w1 = (pos - i0f).astype(corr.dtype)
    i0 = i0f.astype(np.int32)
    i1 = i0 + 1
    m0 = ((i0 >= 0) & (i0 < d_)).astype(corr.dtype)
    m1 = ((i1 >= 0) & (i1 < d_)).astype(corr.dtype)
    idx0 = np.clip(i0, 0, d_ - 1)[None, None]
    idx1 = np.clip(i1, 0, d_ - 1)[None, None]
    g0 = np.take_along_axis(corr, np.broadcast_to(idx0, (b, c, K, h, w)), axis=2)
    g1 = np.take_along_axis(corr, np.broadcast_to(idx1, (b, c, K, h, w)), axis=2)
    return g0 * ((1.0 - w1) * m0)[None, None] + g1 * (w1 * m1)[None, None]


def _run_on_device(corr, md: int, reps: int = 1):
    from concourse.bass_utils import run_bass_kernel_spmd

    K = 2 * md + 1
    nc = _get_nc(md, reps)
    # (B, C, D, H, W) -> (16, D, H, W) -> (16, H, D, W), then pack the
    # diagonal band: xb[p, i, d, m] = corr[p, d, i, d-md+m]
    flat = np.asarray(corr).reshape(PAIRS, D, H, W)
    xt = flat.transpose(0, 2, 1, 3)  # (16, H, D, W) view
    xb = np.zeros((PAIRS, H, D, K), np.float32)
    for d in range(D):
        jlo = max(0, d - md)
        jhi = min(W, d + md + 1)
        mlo = jlo - (d - md)
        xb[:, :, d, mlo : mlo + (jhi - jlo)] = xt[:, :, d, jlo:jhi]
    xb = xb.reshape(PAIRS, H, D * K)
    in_maps = [
        {"x": xb[PAIRS_PER_CORE * c : PAIRS_PER_CORE * (c + 1)]}
        for c in range(N_CORES)
    ]
    res = run_bass_kernel_spmd(nc, in_maps, core_ids=list(range(N_CORES)))
    out = np.concatenate([res.results[c]["y"] for c in range(N_CORES)], axis=0)
    # (16, H, K*W) -> (16, H, K, W) -> (16, K, H, W) -> (B, C, K, H, W)
    out = out.reshape(PAIRS, H, K, W).transpose(0, 2, 1, 3)
    out = np.ascontiguousarray(out).reshape(B, C, K, H, W)
    return out, res


def kernel(corr, maxdisp, is_ux):
    corr = np.asarray(corr)
    md = int(maxdisp)
    ux = int(is_ux)
    if ux != 1 or md < 1 or md > 127 or corr.shape != (B, C, D, H, W):
        return _numpy_ref(corr, md, ux).astype(corr.dtype)
    out, _ = _run_on_device(corr, md)
    return out


# revision 20
# speedup vs baseline: 2.2446x; 1.7057x over previous
"""Corr2Cost sampling kernel for 8 TRN2 NeuronCores.

Math: out[b,c,k,i,j] = lerp of corr[b,c,:,i,j] at depth (j + k - maxdisp)
(is_ux=1) with zero padding outside [0, D-1].  For integer maxdisp the
displacements linspace(-md, md, 2*md+1) are exact integers, so the lerp
weight is exactly 0 and the op is a pure masked integer gather:

    out[b,c,k,i,j] = corr[b,c, j+k-md, i, j]   if 0 <= j+k-md < D else 0

Sharding: data-parallel over the 16 (b,c) pairs -> 2 pairs per core; no
cross-core communication.

Layout strategy (all DMA at max descriptor efficiency):
  - only the band |d - j| <= md of corr is ever read (d = j+k-md, k in
    [0, 2md]), so the host packs the diagonal band per pair:
        xb[i, d*Kb + m] = corr[d, i, d-md+m],  m in [0, Kb), Kb = 2md+1
    (junk where the source j is out of range -- those slots are never
    read).  The per-pair load is then 96 partitions x Kb*D*4 contiguous
    bytes, 21% fewer than the full tensor;
  - the gather for output row k is the constant-stride-Kb slice
        A[i, (j+k-md)*Kb + (2md-k)] -> one strided tensor_copy per k
    into O[i, kk*W + j];
  - O is stored as (i, k, j) slabs (per-partition contiguous), host
    post-transposes to (k, i, j).  Border regions are zeroed by one
    rectangular over-memset per k-chunk;
  - loads go on the SP HWDGE ring (nc.sync), stores on the ACT ring
    (nc.scalar): the two streams are byte-balanced and can overlap.
"""

import numpy as np

B, C, D, H, W = 8, 2, 128, 96, 128
N_CORES = 8
PAIRS = B * C  # 16
PAIRS_PER_CORE = PAIRS // N_CORES  # 2

_NC_CACHE = {}


def _k_chunks(K, small_first=False):
    """Split [0, K) into ~4 chunks; small_first prepends a tiny chunk so the
    pair's first store is ready as soon as its ring frees up."""
    if K < 8:
        return [(0, K)]
    bounds = [0]
    if small_first:
        bounds.append(min(6, K // 2))
    rest = K - bounds[-1]
    n = 4
    start = bounds[-1]
    bounds += [start + round(i * rest / n) for i in range(1, n + 1)]
    return [(bounds[i], bounds[i + 1]) for i in range(len(bounds) - 1)]


def _build_bass(md: int, reps: int = 1):
    """Build + compile the per-core Bass graph for is_ux=1, given maxdisp.

    reps > 1 wraps the body in a hardware For_i loop (timing harness only).
    """
    import concourse.bacc as bacc
    import concourse.mybir as mybir
    import concourse.tile as tile

    K = 2 * md + 1
    f32 = mybir.dt.float32

    nc = bacc.Bacc("TRN2", target_bir_lowering=False, debug=False)
    x = nc.dram_tensor("x", [PAIRS_PER_CORE, H, D * K], f32, kind="ExternalInput")
    y = nc.dram_tensor("y", [PAIRS_PER_CORE, H, K * W], f32, kind="ExternalOutput")

    # Measured on this terminal: only exact-128-partition DMA streams reach
    # ~370 GB/s; 96-partition shapes get ~176, and DMAs running CONCURRENTLY
    # on both HWDGE rings degrade ~2x below running serially on one ring.
    # So: flatten the 2 pairs to 192 rows, tile as 128 + 64 rows, and issue
    # every DMA serially on the single SP ring in stream order.
    ROWS = PAIRS_PER_CORE * H  # 192
    RA = 128                   # rows in the fast tile
    RB = ROWS - RA             # 64

    # copy-group size: k's batched per 3D tensor_copy instruction.  The
    # group reads the union j-window, so up to G-1 diagonal steps land
    # outside the band -- absorbed by PAD junk floats on each side of the
    # tile (values never reach valid output; borders are memset after).
    G = 13
    PAD = (G - 1) * K

    def _groups(k0, k1):
        ks = list(range(k0, k1, G))
        return [(g0, min(g0 + G, k1)) for g0 in ks]

    def body(tc, apool, opool):
        import concourse.bass as bass

        x_flat = x[:].rearrange("p h f -> (p h) f")   # (192, D*K)
        y_flat = y[:].rearrange("p h f -> (p h) f")   # (192, K*W)
        ta = apool.tile([RA, PAD + D * K + PAD], f32)
        tb = apool.tile([RB, PAD + D * K + PAD], f32)
        for t in (ta, tb):
            # pads only absorb junk reads; zero them so nothing is ever
            # read uninitialized (gpsimd is otherwise idle)
            nc.gpsimd.memset(t[:][:, 0:PAD], 0.0)
            nc.gpsimd.memset(t[:][:, PAD + D * K :], 0.0)
        nc.sync.dma_start(out=ta[:][:, PAD : PAD + D * K], in_=x_flat[0:RA])
        nc.sync.dma_start(out=tb[:][:, PAD : PAD + D * K], in_=x_flat[RA:ROWS])
        for a, rows, r0, cp_eng in (
            (ta, RA, 0, nc.vector),
            (tb, RB, RA, nc.vector),
        ):
            a_ap = a[:]
            part_stride = a_ap.ap[0][0]
            for (k0, k1) in _k_chunks(K):
                ck = k1 - k0
                o = opool.tile([rows, ck * W], f32)
                o3 = o[:].rearrange("q (kk j) -> q kk j", j=W)
                for (g0, g1) in _groups(k0, k1):
                    gk = g1 - g0
                    # union j-window over the group's k's
                    jw0 = max(0, md - (g1 - 1))
                    jw1 = min(W - 1, D - 1 + md - g0)
                    wg = jw1 - jw0 + 1
                    # flat band offset for (k=g0, j=jw0), plus left pad
                    off = PAD + (jw0 + g0 - md) * K + (2 * md - g0)
                    base = a_ap[:, off : off + 1]
                    src = bass.AP(
                        base.tensor,
                        base.offset,
                        [[part_stride, rows], [K - 1, gk], [K, wg]],
                    )
                    cp_eng.tensor_copy(
                        o3[:, g0 - k0 : g1 - k0, jw0 : jw1 + 1], src
                    )
                    # the copy wrote zeros into masked cells inside its
                    # window (junk reads hit the zeroed pads); cells outside
                    # the window are all masked -> zero them per group
                    if jw0 > 0:
                        cp_eng.memset(o3[:, g0 - k0 : g1 - k0, 0:jw0], 0.0)
                    if jw1 < W - 1:
                        cp_eng.memset(
                            o3[:, g0 - k0 : g1 - k0, jw1 + 1 : W], 0.0
                        )
                nc.sync.dma_start(
                    out=y_flat[r0 : r0 + rows, k0 * W : k1 * W], in_=o[:]
                )

    with tile.TileContext(nc) as tc:
        with (
            tc.tile_pool(name="a", bufs=1) as apool,
            tc.tile_pool(name="o", bufs=4) as opool,
        ):
            if reps == 1:
                body(tc, apool, opool)
            else:
                with tc.For_i(0, reps, 1):
                    body(tc, apool, opool)

    nc.compile()
    return nc


def _get_nc(md: int, reps: int = 1):
    key = (md, reps)
    if key not in _NC_CACHE:
        _NC_CACHE[key] = _build_bass(md, reps)
    return _NC_CACHE[key]


def _numpy_ref(corr, maxdisp, is_ux):
    """Exact numpy replication of the reference (fallback path)."""
    corr = np.asarray(corr)
    b, c, d_, h, w = corr.shape
    K = 2 * maxdisp + 1
    dx = np.linspace(-float(maxdisp), float(maxdisp), K).astype(np.float32)
    if is_ux:
        base = np.broadcast_to(np.arange(w, dtype=np.float32)[None, :], (h, w))
    else:
        base = np.broadcast_to(np.arange(h, dtype=np.float32)[:, None], (h, w))
    pos = base[None, :, :] + dx[:, None, None]
    i0f = np.floor(pos)
    w1 = (pos - i0f).astype(corr.dtype)
    i0 = i0f.astype(np.int32)
    i1 = i0 + 1
    m0 = ((i0 >= 0) & (i0 < d_)).astype(corr.dtype)
    m1 = ((i1 >= 0) & (i1 < d_)).astype(corr.dtype)
    idx0 = np.clip(i0, 0, d_ - 1)[None, None]
    idx1 = np.clip(i1, 0, d_ - 1)[None, None]
    g0 = np.take_along_axis(corr, np.broadcast_to(idx0, (b, c, K, h, w)), axis=2)
    g1 = np.take_along_axis(corr, np.broadcast_to(idx1, (b, c, K, h, w)), axis=2)
    return g0 * ((1.0 - w1) * m0)[None, None] + g1 * (w1 * m1)[None, None]


def _run_on_device(corr, md: int, reps: int = 1):
    from concourse.bass_utils import run_bass_kernel_spmd

    K = 2 * md + 1
    nc = _get_nc(md, reps)
    # (B, C, D, H, W) -> (16, D, H, W) -> (16, H, D, W), then pack the
    # diagonal band: xb[p, i, d, m] = corr[p, d, i, d-md+m]
    flat = np.asarray(corr).reshape(PAIRS, D, H, W)
    xt = flat.transpose(0, 2, 1, 3)  # (16, H, D, W) view
    xb = np.zeros((PAIRS, H, D, K), np.float32)
    for d in range(D):
        jlo = max(0, d - md)
        jhi = min(W, d + md + 1)
        mlo = jlo - (d - md)
        xb[:, :, d, mlo : mlo + (jhi - jlo)] = xt[:, :, d, jlo:jhi]
    xb = xb.reshape(PAIRS, H, D * K)
    in_maps = [
        {"x": xb[PAIRS_PER_CORE * c : PAIRS_PER_CORE * (c + 1)]}
        for c in range(N_CORES)
    ]
    res = run_bass_kernel_spmd(nc, in_maps, core_ids=list(range(N_CORES)))
    out = np.concatenate([res.results[c]["y"] for c in range(N_CORES)], axis=0)
    # (16, H, K*W) -> (16, H, K, W) -> (16, K, H, W) -> (B, C, K, H, W)
    out = out.reshape(PAIRS, H, K, W).transpose(0, 2, 1, 3)
    out = np.ascontiguousarray(out).reshape(B, C, K, H, W)
    return out, res


def kernel(corr, maxdisp, is_ux):
    corr = np.asarray(corr)
    md = int(maxdisp)
    ux = int(is_ux)
    if ux != 1 or md < 1 or md > 127 or corr.shape != (B, C, D, H, W):
        return _numpy_ref(corr, md, ux).astype(corr.dtype)
    out, _ = _run_on_device(corr, md)
    return out


# revision 21
# speedup vs baseline: 2.5675x; 1.1438x over previous
"""Corr2Cost sampling kernel for 8 TRN2 NeuronCores.

Math: out[b,c,k,i,j] = lerp of corr[b,c,:,i,j] at depth (j + k - maxdisp)
(is_ux=1) with zero padding outside [0, D-1].  For integer maxdisp the
displacements linspace(-md, md, 2*md+1) are exact integers, so the lerp
weight is exactly 0 and the op is a pure masked integer gather:

    out[b,c,k,i,j] = corr[b,c, j+k-md, i, j]   if 0 <= j+k-md < D else 0

Sharding: data-parallel over the 16 (b,c) pairs -> 2 pairs per core; no
cross-core communication.

Layout strategy (all DMA at max descriptor efficiency):
  - only the band |d - j| <= md of corr is ever read (d = j+k-md, k in
    [0, 2md]), so the host packs the diagonal band per pair:
        xb[i, d*Kb + m] = corr[d, i, d-md+m],  m in [0, Kb), Kb = 2md+1
    (junk where the source j is out of range -- those slots are never
    read).  The per-pair load is then 96 partitions x Kb*D*4 contiguous
    bytes, 21% fewer than the full tensor;
  - the gather for output row k is the constant-stride-Kb slice
        A[i, (j+k-md)*Kb + (2md-k)] -> one strided tensor_copy per k
    into O[i, kk*W + j];
  - O is stored as (i, k, j) slabs (per-partition contiguous), host
    post-transposes to (k, i, j).  Border regions are zeroed by one
    rectangular over-memset per k-chunk;
  - loads go on the SP HWDGE ring (nc.sync), stores on the ACT ring
    (nc.scalar): the two streams are byte-balanced and can overlap.
"""

import numpy as np

B, C, D, H, W = 8, 2, 128, 96, 128
N_CORES = 8
PAIRS = B * C  # 16
PAIRS_PER_CORE = PAIRS // N_CORES  # 2

_NC_CACHE = {}


def _k_chunks(K):
    """Split [0, K) into ~3 chunks (store units)."""
    if K < 8:
        return [(0, K)]
    n = 3
    bounds = [round(i * K / n) for i in range(n + 1)]
    return [(bounds[i], bounds[i + 1]) for i in range(n)]


def _build_bass(md: int, reps: int = 1):
    """Build + compile the per-core Bass graph for is_ux=1, given maxdisp.

    reps > 1 wraps the body in a hardware For_i loop (timing harness only).
    """
    import concourse.bacc as bacc
    import concourse.mybir as mybir
    import concourse.tile as tile

    K = 2 * md + 1
    f32 = mybir.dt.float32

    nc = bacc.Bacc("TRN2", target_bir_lowering=False, debug=False)
    x = nc.dram_tensor("x", [PAIRS_PER_CORE, H, D * K], f32, kind="ExternalInput")
    y = nc.dram_tensor("y", [PAIRS_PER_CORE, H, K * W], f32, kind="ExternalOutput")

    # Measured on this terminal: only exact-128-partition DMA streams reach
    # ~370 GB/s; 96-partition shapes get ~176, and DMAs running CONCURRENTLY
    # on both HWDGE rings degrade ~2x below running serially on one ring.
    # So: flatten the 2 pairs to 192 rows, tile as 128 + 64 rows, and issue
    # every DMA serially on the single SP ring in stream order.
    ROWS = PAIRS_PER_CORE * H  # 192
    RA = 128                   # rows in the fast tile
    RB = ROWS - RA             # 64

    # copy-group size: k's batched per 3D tensor_copy instruction.  The
    # group reads the union j-window, so up to G-1 diagonal steps land
    # outside the band -- absorbed by PAD junk floats on each side of the
    # tile (values never reach valid output; borders are memset after).
    G = 17
    PAD = (G - 1) * K

    def _groups(k0, k1):
        ks = list(range(k0, k1, G))
        return [(g0, min(g0 + G, k1)) for g0 in ks]

    def body(tc, apool, opool):
        import concourse.bass as bass

        x_flat = x[:].rearrange("p h f -> (p h) f")   # (192, D*K)
        y_flat = y[:].rearrange("p h f -> (p h) f")   # (192, K*W)
        ta = apool.tile([RA, PAD + D * K + PAD], f32)
        tb = apool.tile([RB, PAD + D * K + PAD], f32)
        for t in (ta, tb):
            # pads only absorb junk reads; zero them so nothing is ever
            # read uninitialized (gpsimd is otherwise idle)
            nc.gpsimd.memset(t[:][:, 0:PAD], 0.0)
            nc.gpsimd.memset(t[:][:, PAD + D * K :], 0.0)
        nc.sync.dma_start(out=ta[:][:, PAD : PAD + D * K], in_=x_flat[0:RA])
        nc.sync.dma_start(out=tb[:][:, PAD : PAD + D * K], in_=x_flat[RA:ROWS])
        for a, rows, r0, cp_eng in (
            (ta, RA, 0, nc.vector),
            (tb, RB, RA, nc.vector),
        ):
            a_ap = a[:]
            part_stride = a_ap.ap[0][0]
            for (k0, k1) in _k_chunks(K):
                ck = k1 - k0
                o = opool.tile([rows, ck * W], f32)
                o3 = o[:].rearrange("q (kk j) -> q kk j", j=W)
                for (g0, g1) in _groups(k0, k1):
                    gk = g1 - g0
                    # union j-window over the group's k's
                    jw0 = max(0, md - (g1 - 1))
                    jw1 = min(W - 1, D - 1 + md - g0)
                    wg = jw1 - jw0 + 1
                    # flat band offset for (k=g0, j=jw0), plus left pad
                    off = PAD + (jw0 + g0 - md) * K + (2 * md - g0)
                    base = a_ap[:, off : off + 1]
                    src = bass.AP(
                        base.tensor,
                        base.offset,
                        [[part_stride, rows], [K - 1, gk], [K, wg]],
                    )
                    cp_eng.tensor_copy(
                        o3[:, g0 - k0 : g1 - k0, jw0 : jw1 + 1], src
                    )
                    # the copy wrote zeros into masked cells inside its
                    # window (junk reads hit the zeroed pads); cells outside
                    # the window are all masked -> zero them per group
                    if jw0 > 0:
                        cp_eng.memset(o3[:, g0 - k0 : g1 - k0, 0:jw0], 0.0)
                    if jw1 < W - 1:
                        cp_eng.memset(
                            o3[:, g0 - k0 : g1 - k0, jw1 + 1 : W], 0.0
                        )
                nc.sync.dma_start(
                    out=y_flat[r0 : r0 + rows, k0 * W : k1 * W], in_=o[:]
                )

    with tile.TileContext(nc) as tc:
        with (
            tc.tile_pool(name="a", bufs=1) as apool,
            tc.tile_pool(name="o", bufs=4) as opool,
        ):
            if reps == 1:
                body(tc, apool, opool)
            else:
                with tc.For_i(0, reps, 1):
                    body(tc, apool, opool)

    nc.compile()
    return nc


def _get_nc(md: int, reps: int = 1):
    key = (md, reps)
    if key not in _NC_CACHE:
        _NC_CACHE[key] = _build_bass(md, reps)
    return _NC_CACHE[key]


def _numpy_ref(corr, maxdisp, is_ux):
    """Exact numpy replication of the reference (fallback path)."""
    corr = np.asarray(corr)
    b, c, d_, h, w = corr.shape
    K = 2 * maxdisp + 1
    dx = np.linspace(-float(maxdisp), float(maxdisp), K).astype(np.float32)
    if is_ux:
        base = np.broadcast_to(np.arange(w, dtype=np.float32)[None, :], (h, w))
    else:
        base = np.broadcast_to(np.arange(h, dtype=np.float32)[:, None], (h, w))
    pos = base[None, :, :] + dx[:, None, None]
    i0f = np.floor(pos)
    w1 = (pos - i0f).astype(corr.dtype)
    i0 = i0f.astype(np.int32)
    i1 = i0 + 1
    m0 = ((i0 >= 0) & (i0 < d_)).astype(corr.dtype)
    m1 = ((i1 >= 0) & (i1 < d_)).astype(corr.dtype)
    idx0 = np.clip(i0, 0, d_ - 1)[None, None]
    idx1 = np.clip(i1, 0, d_ - 1)[None, None]
    g0 = np.take_along_axis(corr, np.broadcast_to(idx0, (b, c, K, h, w)), axis=2)
    g1 = np.take_along_axis(corr, np.broadcast_to(idx1, (b, c, K, h, w)), axis=2)
    return g0 * ((1.0 - w1) * m0)[None, None] + g1 * (w1 * m1)[None, None]


def _run_on_device(corr, md: int, reps: int = 1):
    from concourse.bass_utils import run_bass_kernel_spmd

    K = 2 * md + 1
    nc = _get_nc(md, reps)
    # (B, C, D, H, W) -> (16, D, H, W) -> (16, H, D, W), then pack the
    # diagonal band: xb[p, i, d, m] = corr[p, d, i, d-md+m]
    flat = np.asarray(corr).reshape(PAIRS, D, H, W)
    xt = flat.transpose(0, 2, 1, 3)  # (16, H, D, W) view
    xb = np.zeros((PAIRS, H, D, K), np.float32)
    for d in range(D):
        jlo = max(0, d - md)
        jhi = min(W, d + md + 1)
        mlo = jlo - (d - md)
        xb[:, :, d, mlo : mlo + (jhi - jlo)] = xt[:, :, d, jlo:jhi]
    xb = xb.reshape(PAIRS, H, D * K)
    in_maps = [
        {"x": xb[PAIRS_PER_CORE * c : PAIRS_PER_CORE * (c + 1)]}
        for c in range(N_CORES)
    ]
    res = run_bass_kernel_spmd(nc, in_maps, core_ids=list(range(N_CORES)))
    out = np.concatenate([res.results[c]["y"] for c in range(N_CORES)], axis=0)
    # (16, H, K*W) -> (16, H, K, W) -> (16, K, H, W) -> (B, C, K, H, W)
    out = out.reshape(PAIRS, H, K, W).transpose(0, 2, 1, 3)
    out = np.ascontiguousarray(out).reshape(B, C, K, H, W)
    return out, res


def kernel(corr, maxdisp, is_ux):
    corr = np.asarray(corr)
    md = int(maxdisp)
    ux = int(is_ux)
    if ux != 1 or md < 1 or md > 127 or corr.shape != (B, C, D, H, W):
        return _numpy_ref(corr, md, ux).astype(corr.dtype)
    out, _ = _run_on_device(corr, md)
    return out
